# revision 73
# baseline (speedup 1.0000x reference)
"""Fused transformer block (LN1 -> 16-head causal attention -> LN2 -> FFN,
two residuals) on 8 Trainium2 NeuronCores.

Sharding strategy
-----------------
- Attention is head-parallel: core c owns heads (2c, 2c+1) and computes their
  Q^T/K^T/V^T and attention output O^T for ALL 4096 tokens (B*T flattened).
- LN1 + transpose of the normalized activations is sequence-parallel (512
  tokens per core), stitched with an AllGather of x^T (feature-major).
- An AllToAll converts the head-parallel attention output into token-parallel
  layout; residual + LN2 + the whole FFN then run sequence-parallel (512
  tokens per core) with zero further communication. Each core returns its
  512-token slice of the output; the host concatenates.

Performance notes:
- LN gains/biases are folded into the weights host-side: Wq/Wk/Wv absorb
  ln1_g (per-head bias vectors absorb ln1_b), W1 absorbs ln2_g and b1
  absorbs ln2_b@W1. LN on device is just (x - mean) * rstd.
- The gathered x^T is read back in ONE 3-D DMA per 512-token chunk; W1 is
  preloaded in 8 row-block DMAs right after the AllToAll completes; the
  output is accumulated in SBUF and written in 4 DMAs.
- Attention is software-pipelined across chunks: scores+exp of chunk c+1 are
  emitted before the PV accumulation of chunk c, so the PE always has
  runnable work while the Scalar engine drains the softmax exps. Two score
  tiles share a 2-bank PSUM tile so one Exp covers 1024 columns.
- The attention-output AllToAll travels in bf16 (halves the exchange).
"""

import sys

if "/opt/trn_rl_repo" not in sys.path:
    sys.path.insert(0, "/opt/trn_rl_repo")

import ml_dtypes
import numpy as np

import concourse.bass as bass
import concourse.mybir as mybir
import concourse.tile as tile
from concourse import bacc
from concourse.bass import ds, ts
from concourse.masks import make_identity

# ── Problem constants (hardcoded; see spec) ──────────────────────────────────
B, T, D = 2, 2048, 1024
H, HS = 16, 64
FF = 4 * D  # 4096
EPS = 1e-5
NCORES = 8
NT = B * T  # 4096 tokens
TC = NT // NCORES  # 512 tokens per core
NTT = TC // 128  # 4 token tiles per core
HPC = H // NCORES  # 2 heads per core
SCALE = 1.0 / float(np.sqrt(D))  # reference scales scores by D**-0.5

F32 = mybir.dt.float32
BF16 = mybir.dt.bfloat16
AF = mybir.ActivationFunctionType
OP = mybir.AluOpType


def build():
    nc = bacc.Bacc(num_devices=NCORES)

    emb = nc.dram_tensor("emb", [TC, D], F32, kind="ExternalInput")
    # per-core head slice of Wq/Wk/Wv (ln1_g folded in), pre-rearranged to
    # [128(d_in), 8(d_tile), 128(2*HS)]
    wq = nc.dram_tensor("wq", [128, 8, 128], BF16, kind="ExternalInput")
    wk = nc.dram_tensor("wk", [128, 8, 128], BF16, kind="ExternalInput")
    wv = nc.dram_tensor("wv", [128, 8, 128], BF16, kind="ExternalInput")
    # per-core QKV bias columns (ln1_b folded through the projections)
    bqkv = nc.dram_tensor("bqkv", [128, 3], F32, kind="ExternalInput")
    w1 = nc.dram_tensor("w1", [D, FF], BF16, kind="ExternalInput")
    w2 = nc.dram_tensor("w2", [FF, D], BF16, kind="ExternalInput")
    b1r = nc.dram_tensor("b1r", [128, FF // 128], F32, kind="ExternalInput")
    b2f = nc.dram_tensor("b2f", [D], F32, kind="ExternalInput")
    out = nc.dram_tensor("out", [TC, D], F32, kind="ExternalOutput")

    rg = [list(range(NCORES))]

    with tile.TileContext(nc) as tc:
        # Pools are allocated just-in-time and released LIFO per (space, side).
        const = tc.alloc_tile_pool(name="const", bufs=1)
        dram = tc.alloc_tile_pool(name="dram", bufs=1, space="DRAM")
        stat = tc.alloc_tile_pool(name="stat", bufs=4)
        xln = tc.alloc_tile_pool(name="xln", bufs=2)
        embs = tc.alloc_tile_pool(name="embs", bufs=2, side="right")

        identity = const.tile([128, 128], F32, name="identity")
        make_identity(nc, identity)
        identity_b = const.tile([128, 128], BF16, name="identity_b")
        nc.vector.tensor_copy(out=identity_b, in_=identity)
        eps_t = const.tile([128, 1], F32, name="eps_t")
        nc.vector.memset(eps_t, EPS)
        b1s = const.tile([128, FF // 128], F32, name="b1s")
        nc.sync.dma_start(out=b1s, in_=b1r[:, :])
        b2b = const.tile([128, D], F32, name="b2b")
        b2a = b2f[:]
        nc.sync.dma_start(
            out=b2b,
            in_=bass.AP(tensor=b2a.tensor, offset=b2a.offset, ap=[[0, 128], [1, D]]),
        )
        bqs = const.tile([128, 3], F32, name="bqs")
        nc.sync.dma_start(out=bqs, in_=bqkv[:, :])

        # DRAM bounce buffers
        cc_x_in = dram.tile([D, TC], BF16, name="cc_x_in")
        xg = dram.tile([NCORES * D, TC], BF16, name="xg", addr_space="Shared")
        cc_a_in = dram.tile([NCORES, 130, TC], BF16, name="cc_a_in")
        cc_a_out = dram.tile([NCORES, 130, TC], BF16, name="cc_a_out")

        # attention-lifetime pools (released after the AllToAll)
        qkres = tc.alloc_tile_pool(name="qkres", bufs=1)
        vsbp = tc.alloc_tile_pool(name="vsbp", bufs=1)
        otp = tc.alloc_tile_pool(name="otp", bufs=3)
        ptp = tc.alloc_tile_pool(name="ptp", bufs=6)
        attnc = tc.alloc_tile_pool(name="attnc", bufs=1)

        qT = qkres.tile([128, NT], BF16, name="qT")
        kT = qkres.tile([128, NT], BF16, name="kT")
        v_sb = vsbp.tile([128, NT // 128, HPC, 65], BF16, name="v_sb")
        ones_f = attnc.tile([128, 64], F32, name="ones_f")
        nc.vector.memset(ones_f, 1.0)

        # QKV-lifetime pools
        vtp = tc.alloc_tile_pool(name="vtp", bufs=1)
        wqkvc = tc.alloc_tile_pool(name="wqkvc", bufs=1)
        xrhs = tc.alloc_tile_pool(name="xrhs", bufs=2)
        vT = vtp.tile([128, NT], BF16, name="vT")
        wqs = wqkvc.tile([128, 8, 128], BF16, name="wqs")
        nc.sync.dma_start(out=wqs, in_=wq[:, :, :])
        wks = wqkvc.tile([128, 8, 128], BF16, name="wks")
        nc.sync.dma_start(out=wks, in_=wk[:, :, :])
        wvs = wqkvc.tile([128, 8, 128], BF16, name="wvs")
        nc.sync.dma_start(out=wvs, in_=wv[:, :, :])

        def layer_norm(src_tile, dst_tile, use_scalar=False):
            """dst = (src - mean) * rstd  (per 128-token tile, stats over D).

            LN gain/bias are folded into the downstream weights host-side.
            With use_scalar the wide normalize pass runs on the Scalar engine
            as Identity(src * rstd + (-mean * rstd)), halving the Vector load.
            """
            st = stat.tile([128, 2, 6], F32, name="st", tag="st")
            nc.vector.bn_stats(out=st[:, 0, :], in_=src_tile[:, 0:512])
            nc.vector.bn_stats(out=st[:, 1, :], in_=src_tile[:, 512:1024])
            mv = stat.tile([128, 2], F32, name="mv", tag="mv")
            nc.vector.bn_aggr(out=mv, in_=st)
            std = stat.tile([128, 1], F32, name="std", tag="std")
            nc.scalar.activation(
                out=std, in_=mv[:, 1:2], func=AF.Sqrt, bias=eps_t, scale=1.0
            )
            rstd = stat.tile([128, 1], F32, name="rstd", tag="rstd")
            nc.vector.reciprocal(out=rstd, in_=std)
            if use_scalar:
                nm = stat.tile([128, 1], F32, name="nm", tag="nm")
                nc.vector.tensor_scalar(
                    out=nm,
                    in0=mv[:, 0:1],
                    scalar1=rstd,
                    scalar2=-1.0,
                    op0=OP.mult,
                    op1=OP.mult,
                )
                nc.scalar.activation(
                    out=dst_tile, in_=src_tile, func=AF.Identity, bias=nm, scale=rstd
                )
            else:
                nc.vector.tensor_scalar(
                    out=dst_tile,
                    in0=src_tile,
                    scalar1=mv[:, 0:1],
                    scalar2=rstd,
                    op0=OP.subtract,
                    op1=OP.mult,
                )

        # ── Phase A+B: LN1 on own 512-token chunk, transpose, send to AG ──
        xtp = tc.alloc_tile_pool(name="xtp", bufs=1)
        ab_tp = tc.alloc_tile_pool(name="ab_tp", bufs=4, space="PSUM")
        xt_tiles = [xtp.tile([128, TC], BF16, name=f"xt{k}") for k in range(8)]
        xn_tiles = []
        for i in range(NTT):
            et = embs.tile([128, D], F32, name="et", tag="et", bufs=4)
            nc.sync.dma_start(out=et, in_=emb[ts(i, 128), :])
            xn = xln.tile([128, D], F32, name="xn", tag="xn", bufs=4)
            layer_norm(et, xn, use_scalar=(i % 2 == 1))
            xn_tiles.append(xn)
        # k-major transposes with a per-block DMA: each feature block of the
        # AllGather input ships as soon as it is complete, so the collective's
        # data dependency clears right after the last block instead of after
        # a bulk 8-DMA tail
        for k in range(8):
            for i in range(NTT):
                ps = ab_tp.tile([128, 128], F32, name="abtp", tag="abtp")
                nc.tensor.transpose(ps, xn_tiles[i][:, ts(k, 128)], identity)
                if k % 2 == 0:
                    nc.vector.tensor_copy(out=xt_tiles[k][:, ts(i, 128)], in_=ps)
                else:
                    nc.scalar.copy(out=xt_tiles[k][:, ts(i, 128)], in_=ps)
            nc.sync.dma_start(out=cc_x_in[ts(k, 128), :], in_=xt_tiles[k])

        # ── Phase C: AllGather x^T chunks ───────────────────────────────
        nc.gpsimd.collective_compute(
            "AllGather", OP.bypass, replica_groups=rg, ins=[cc_x_in.opt()], outs=[xg.opt()]
        )
        xtp.release()
        ab_tp.release()

        # causal masks (emitted after the collective so GpSimd reaches the
        # collective trigger as early as possible)
        masks = []
        for i in range(4):
            m = attnc.tile([128, TC], BF16, name=f"mask{i}")
            nc.gpsimd.memset(m, 1.0)
            # keep 1 where (j - i_partition - 128*i) >= 0, else 0
            nc.gpsimd.affine_select(
                out=m,
                in_=m,
                compare_op=OP.is_ge,
                fill=0.0,
                base=-128 * i,
                pattern=[[1, TC]],
                channel_multiplier=-1,
            )
            masks.append(m)

        # ── Phase D: Q^T/K^T/V^T for own 2 heads over all 4096 tokens ──
        qkv_ps = tc.alloc_tile_pool(name="qkv_ps", bufs=2, space="PSUM")
        for cb in range(NCORES):
            xr = xrhs.tile([128, 8, TC], BF16, name="xr", tag="xr")
            xga = xg[:]
            if cb == 0:
                # first chunk: per-block DMAs so the k=0 matmul starts as
                # soon as its 128 KB lands instead of after the full MB
                for k in range(8):
                    src = bass.AP(
                        tensor=xga.tensor,
                        offset=xga.offset + 128 * k * TC,
                        ap=[[TC, 128], [1, TC]],
                    )
                    nc.sync.dma_start(out=xr[:, k, :], in_=src)
            else:
                # one 3-D DMA brings in all 8 feature blocks of this chunk
                src = bass.AP(
                    tensor=xga.tensor,
                    offset=xga.offset + D * cb * TC,
                    ap=[[TC, 128], [128 * TC, 8], [1, TC]],
                )
                nc.sync.dma_start(out=xr, in_=src)
            pq = qkv_ps.tile([128, TC], F32, name="pq", tag="pq")
            pk = qkv_ps.tile([128, TC], F32, name="pk", tag="pk")
            pv = qkv_ps.tile([128, TC], F32, name="pv", tag="pv")
            for k in range(8):
                kw = dict(start=(k == 0), stop=(k == 7))
                nc.tensor.matmul(pq, lhsT=wqs[:, k, :], rhs=xr[:, k, :], **kw)
                nc.tensor.matmul(pk, lhsT=wks[:, k, :], rhs=xr[:, k, :], **kw)
                nc.tensor.matmul(pv, lhsT=wvs[:, k, :], rhs=xr[:, k, :], **kw)
            nc.vector.tensor_scalar_add(
                out=qT[:, ts(cb, TC)], in0=pq, scalar1=bqs[:, 0:1]
            )
            nc.vector.tensor_scalar_add(
                out=kT[:, ts(cb, TC)], in0=pk, scalar1=bqs[:, 1:2]
            )
            nc.scalar.activation(
                out=vT[:, ts(cb, TC)], in_=pv, func=AF.Identity,
                bias=bqs[:, 2:3], scale=1.0,
            )

        # ── Phase E: V^T -> natural V per s-tile, ones-augmented ────────
        for st_ in range(NT // 128):
            ps = qkv_ps.tile([128, 128], BF16, name="tpb", tag="tpb")
            nc.tensor.transpose(ps, vT[:, ts(st_, 128)], identity_b)
            nc.vector.tensor_copy(
                out=v_sb[:, st_, :, 0:64],
                in_=ps.rearrange("p (h e) -> p h e", h=HPC),
            )
        nc.vector.tensor_copy(
            out=v_sb[:, :, :, 64:65],
            in_=ones_f[:, 0:64].rearrange("p (a b c) -> p a b c", a=NT // 128, b=HPC),
        )
        xrhs.release()
        wqkvc.release()
        vtp.release()
        qkv_ps.release()

        # ── Phase F: causal attention, software-pipelined across chunks ──
        # Per chunk: S^T matmuls stream through PSUM banks, exp evacuates to
        # bf16 SBUF; causal masking is a binary multiply on the exp tiles.
        # The scores+exp of chunk c+1 are emitted BEFORE the PV accumulation
        # of chunk c so the PE has dependency-free work while Scalar drains.
        s_ps = tc.alloc_tile_pool(name="s_ps", bufs=3, space="PSUM")
        o_ps = tc.alloc_tile_pool(name="o_ps", bufs=1, space="PSUM")

        def emit_scores(gc):
            b = gc // 4
            lc = gc % 4
            nst = 4 * lc + 4  # s-tiles (128 wide) within this batch
            t0g = gc * TC
            pts = {}
            # two s-tiles share one 2-bank PSUM tile so a single Exp covers
            # 1024 columns (halves the Scalar per-instruction overhead)
            # Causal column-skipping: a diag-d tile's first 128*d query
            # columns never survive the mask, so every stage (scores, exp,
            # mask, PV) is restricted to the columns the next stage reads.
            for sp in range(nst // 2):
                d0 = 2 * sp - 4 * lc
                d1 = d0 + 1
                cl = 128 * d0 if d0 > 0 else 0  # pair-wide exp start column
                for h in range(HPC):
                    ps_ = s_ps.tile([128, 2, TC], F32, name="ps_", tag="ps_")
                    pt_ = ptp.tile([128, 2, TC], BF16, name="pt_", tag="pt_", bufs=30)
                    for hf in range(2):
                        stl = 2 * sp + hf
                        sg = b * 16 + stl
                        diag = stl - 4 * lc
                        c0 = 128 * diag if diag > 0 else 0
                        # heads use PE row-groups 0-63 / 64-127 -> concurrent
                        nc.tensor.matmul(
                            ps_[:, hf, ds(c0, TC - c0)],
                            lhsT=kT[ts(h, 64), ts(sg, 128)],
                            rhs=qT[ts(h, 64), ds(t0g + c0, TC - c0)],
                            start=True,
                            stop=True,
                        )
                    nc.scalar.activation(
                        out=pt_[:, :, ds(cl, TC - cl)],
                        in_=ps_[:, :, ds(cl, TC - cl)],
                        func=AF.Exp,
                        scale=SCALE,
                    )
                    for hf in range(2):
                        stl = 2 * sp + hf
                        diag = stl - 4 * lc
                        c0 = 128 * diag if diag > 0 else 0
                        if diag >= 0:
                            nc.vector.tensor_mul(
                                out=pt_[:, hf, ds(c0, TC - c0)],
                                in0=pt_[:, hf, ds(c0, TC - c0)],
                                in1=masks[diag][:, ds(c0, TC - c0)],
                            )
                        pts[(stl, h)] = (pt_, hf, c0)
            return pts

        def emit_pv(gc, pts):
            b = gc // 4
            lc = gc % 4
            nst = 4 * lc + 4
            for h in range(HPC):
                po = o_ps.tile([65, TC], F32, name=f"po{h}", tag=f"po{h}", bufs=1)
                for stl in range(nst):
                    sg = b * 16 + stl
                    pt_, hf, c0 = pts[(stl, h)]
                    nc.tensor.matmul(
                        po[:, ds(c0, TC - c0)],
                        lhsT=v_sb[:, sg, h, :],
                        rhs=pt_[:, hf, ds(c0, TC - c0)],
                        start=(stl == 0),
                        stop=(stl == nst - 1),
                        skip_group_check=(c0 > 0),
                    )
                # one copy carries the 64 output rows AND the denominator row;
                # two DMAs place them in the unchanged AllToAll layout
                o65 = otp.tile([65, TC], BF16, name="o65", tag=f"o65{h}", bufs=2)
                nc.vector.tensor_copy(out=o65, in_=po)
                nc.sync.dma_start(
                    out=cc_a_in[gc, ds(64 * h, 64), :], in_=o65[0:64, :]
                )
                nc.sync.dma_start(
                    out=cc_a_in[gc, 128 + h : 129 + h, :], in_=o65[64:65, :]
                )

        # residual emb tiles for phase H: loaded during attention so the
        # DMAs don't contend with the AllToAll
        et_tiles = []
        for i in range(NTT):
            et = embs.tile([128, D], F32, name="et", tag=f"et{i}", bufs=1)
            nc.sync.dma_start(out=et, in_=emb[ts(i, 128), :])
            et_tiles.append(et)

        prev = None
        for gc in range(NCORES):
            pts = emit_scores(gc)
            if prev is not None:
                emit_pv(prev[0], prev[1])
            prev = (gc, pts)
        emit_pv(prev[0], prev[1])

        # ── Phase G: AllToAll -> unnormalized attn^T + denoms, own tokens ──
        nc.gpsimd.collective_compute(
            "AllToAll", OP.bypass, replica_groups=rg, ins=[cc_a_in.opt()], outs=[cc_a_out.opt()]
        )
        o_ps.release()
        s_ps.release()
        attnc.release()
        ptp.release()
        otp.release()
        vsbp.release()
        qkres.release()

        # ── Phase H: normalize + attn residual + LN2, y -> y^T ──────────
        asbp = tc.alloc_tile_pool(name="asbp", bufs=4, side="right")
        h_tp = tc.alloc_tile_pool(name="h_tp", bufs=5, space="PSUM")
        x2p = tc.alloc_tile_pool(name="x2p", bufs=1)
        ytp = tc.alloc_tile_pool(name="ytp", bufs=1)
        w1p = tc.alloc_tile_pool(name="w1p", bufs=1)
        x2_tiles = [x2p.tile([128, D], F32, name=f"x2_{i}") for i in range(NTT)]
        yt_tiles = [ytp.tile([128, TC], BF16, name=f"yt{k}") for k in range(8)]
        w1sb = [w1p.tile([128, FF], BF16, name=f"w1sb{k}") for k in range(8)]
        asb_tiles = []
        for c in range(NCORES):
            asb = asbp.tile([128, TC], BF16, name="asb", tag="asb", bufs=8)
            nc.sync.dma_start(out=asb, in_=cc_a_out[c, 0:128, :])
            dnm = asbp.tile([2, TC], BF16, name="dnm", tag="dnm", bufs=8)
            nc.sync.dma_start(out=dnm, in_=cc_a_out[c, 128:130, :])
            asb_tiles.append((asb, dnm))
            if c == 0:
                # W1 preload: issued on Sync right after the first post-A2A
                # load so the 8 MB of reads don't contend with the collective
                for k in range(8):
                    nc.sync.dma_start(out=w1sb[k], in_=w1[ts(k, 128), :])
        # i-major: each x2 tile completes as early as possible so its LN2
        # (emitted right after) overlaps the remaining residual work
        yn_tiles = []
        for i in range(NTT):
            for c in range(NCORES):
                asb, dnm = asb_tiles[c]
                pn = h_tp.tile([128, 128], BF16, name="htp", tag="htp")
                nc.tensor.transpose(pn, asb[:, ts(i, 128)], identity_b)
                pd = h_tp.tile([128, 2], BF16, name="hpd", tag="htp")
                nc.tensor.transpose(pd, dnm[:, ts(i, 128)], identity_b[0:2, 0:2])
                rcp = asbp.tile([128, 2], F32, name="rcp", tag="rcp")
                nc.vector.reciprocal(out=rcp, in_=pd)
                for h in range(HPC):
                    nc.vector.scalar_tensor_tensor(
                        out=x2_tiles[i][:, ds(128 * c + 64 * h, 64)],
                        in0=pn[:, ts(h, 64)],
                        scalar=rcp[:, h : h + 1],
                        in1=et_tiles[i][:, ds(128 * c + 64 * h, 64)],
                        op0=OP.mult,
                        op1=OP.add,
                    )
            yn = xln.tile([128, D], F32, name="yn", tag="yn", bufs=4)
            layer_norm(x2_tiles[i], yn, use_scalar=(i % 2 == 1))
            yn_tiles.append(yn)
        # k-major transposes: yt[0] (which gates the FFN's first matmul)
        # completes first instead of last
        for k in range(8):
            for i in range(NTT):
                ps = h_tp.tile([128, 128], F32, name="htp2", tag="htp2", bufs=3)
                nc.tensor.transpose(ps, yn_tiles[i][:, ts(k, 128)], identity)
                if k % 2 == 0:
                    nc.vector.tensor_copy(out=yt_tiles[k][:, ts(i, 128)], in_=ps)
                else:
                    nc.scalar.copy(out=yt_tiles[k][:, ts(i, 128)], in_=ps)
        asbp.release()
        embs.release()
        h_tp.release()

        # ── Phase J: FFN up-projection, h^T = relu(W1^T y^T + b1) ───────
        htp = tc.alloc_tile_pool(name="htp", bufs=1)
        w2sp = tc.alloc_tile_pool(name="w2sp", bufs=6)
        outsp = tc.alloc_tile_pool(name="outs", bufs=1)
        h_ps = tc.alloc_tile_pool(name="h_ps", bufs=4, space="PSUM")
        ht_tiles = [htp.tile([128, TC], BF16, name=f"ht{j}") for j in range(FF // 128)]
        out_sb = [outsp.tile([128, D], F32, name=f"osb{i}") for i in range(NTT)]
        # fold the down-projection bias into the residual while Vector is idle
        # (LN2 has already consumed x2, so this is safe)
        for i in range(NTT):
            nc.vector.tensor_add(out=x2_tiles[i], in0=x2_tiles[i], in1=b2b)
        for jg in range(16):
            phs = [h_ps.tile([128, TC], F32, name=f"ph{jj}", tag="ph") for jj in range(2)]
            for k in range(8):
                for jj in range(2):
                    nc.tensor.matmul(
                        phs[jj],
                        lhsT=w1sb[k][:, ds(256 * jg + 128 * jj, 128)],
                        rhs=yt_tiles[k],
                        start=(k == 0),
                        stop=(k == 7),
                    )
            for jj in range(2):
                jt = 2 * jg + jj
                nc.scalar.activation(
                    out=ht_tiles[jt],
                    in_=phs[jj],
                    func=AF.Relu,
                    bias=b1s[:, jt : jt + 1],
                    scale=1.0,
                )
        h_ps.release()

        # ── Phase K: FFN down-projection, natural [token, D] accumulation ──
        # lhsT is an h^T chunk reused for both 512-wide halves of W2's rows;
        # each token tile owns a 2-bank PSUM accumulator, so the output needs
        # no final transposes — just one residual add per tile.
        f_ps = tc.alloc_tile_pool(name="f_ps", bufs=4, space="PSUM")
        pfs = [f_ps.tile([128, D], F32, name=f"pf{i}", tag="pf") for i in range(NTT)]
        for jt in range(FF // 128):
            w2t = w2sp.tile([128, D], BF16, name="w2t", tag="w2t")
            nc.sync.dma_start(out=w2t, in_=w2[ts(jt, 128), :])
            for i in range(NTT):
                for dh in range(2):
                    nc.tensor.matmul(
                        pfs[i][:, ts(dh, 512)],
                        lhsT=ht_tiles[jt][:, ts(i, 128)],
                        rhs=w2t[:, ts(dh, 512)],
                        start=(jt == 0),
                        stop=(jt == FF // 128 - 1),
                    )
        for i in range(NTT):
            # half-width adds + writes so the output DMA starts draining
            # while the second half is still being summed
            for dh in range(2):
                nc.vector.tensor_add(
                    out=out_sb[i][:, ts(dh, 512)],
                    in0=pfs[i][:, ts(dh, 512)],
                    in1=x2_tiles[i][:, ts(dh, 512)],
                )
                nc.sync.dma_start(
                    out=out[ts(i, 128), ds(512 * dh, 512)],
                    in_=out_sb[i][:, ts(dh, 512)],
                )

        f_ps.release()
        outsp.release()
        w2sp.release()
        htp.release()
        w1p.release()
        ytp.release()
        x2p.release()
        xln.release()
        stat.release()
        dram.release()
        const.release()
    nc.finalize()
    return nc


_NC = None


def _get_nc():
    global _NC
    if _NC is None:
        _NC = build()
    return _NC


def make_in_maps(embds, Wq, Wk, Wv, ln1_g, ln1_b, ln2_g, ln2_b, W1, b1, W2, b2):
    embds = np.ascontiguousarray(np.asarray(embds, dtype=np.float32)).reshape(NT, D)
    Wq = np.asarray(Wq, dtype=np.float32)
    Wk = np.asarray(Wk, dtype=np.float32)
    Wv = np.asarray(Wv, dtype=np.float32)
    W1 = np.ascontiguousarray(np.asarray(W1, dtype=np.float32))
    W2 = np.ascontiguousarray(np.asarray(W2, dtype=np.float32))
    b1 = np.asarray(b1, dtype=np.float32)
    b2 = np.asarray(b2, dtype=np.float32)
    g1 = np.asarray(ln1_g, dtype=np.float32)
    bb1 = np.asarray(ln1_b, dtype=np.float32)
    g2 = np.asarray(ln2_g, dtype=np.float32)
    bb2 = np.asarray(ln2_b, dtype=np.float32)

    # Fold LN1 gain/bias into the QKV projections:
    #   q = (xn*g1 + b1) @ Wq = xn @ (g1[:,None]*Wq) + b1@Wq
    Wqf = Wq * g1[None, :, None]
    Wkf = Wk * g1[None, :, None]
    Wvf = Wv * g1[None, :, None]
    bq = np.einsum("d,hde->he", bb1, Wq)  # [H, HS]
    bk = np.einsum("d,hde->he", bb1, Wk)
    bv = np.einsum("d,hde->he", bb1, Wv)

    # Fold LN2 gain/bias into the FFN up-projection:
    #   h_pre = (yn*g2 + b2ln) @ W1 + b1 = yn @ (g2[:,None]*W1) + (b2ln@W1 + b1)
    W1f = (W1 * g2[:, None]).astype(ml_dtypes.bfloat16)
    b1f = b1 + bb2 @ W1
    W2b = W2.astype(ml_dtypes.bfloat16)
    b1r = np.ascontiguousarray(b1f.reshape(FF // 128, 128).T.astype(np.float32))

    def _w_slice(W, c):
        # heads (2c, 2c+1): [2, D, HS] -> [D, 2*HS] -> [128, 8, 128]
        s = W[2 * c : 2 * c + 2].transpose(1, 0, 2).reshape(D, 2 * HS)
        r = np.ascontiguousarray(s.reshape(8, 128, 2 * HS).transpose(1, 0, 2))
        return r.astype(ml_dtypes.bfloat16)

    in_maps = []
    for c in range(NCORES):
        bqkv = np.stack(
            [
                np.concatenate([bq[2 * c], bq[2 * c + 1]]),
                np.concatenate([bk[2 * c], bk[2 * c + 1]]),
                np.concatenate([bv[2 * c], bv[2 * c + 1]]),
            ],
            axis=1,
        ).astype(np.float32)  # [128, 3]
        in_maps.append(
            {
                "emb": np.ascontiguousarray(embds[c * TC : (c + 1) * TC]),
                "wq": _w_slice(Wqf, c),
                "wk": _w_slice(Wkf, c),
                "wv": _w_slice(Wvf, c),
                "bqkv": np.ascontiguousarray(bqkv),
                "w1": W1f,
                "w2": W2b,
                "b1r": b1r,
                "b2f": np.ascontiguousarray(b2),
            }
        )
    return in_maps


def run(in_maps, trace=False, **kwargs):
    from concourse.bass_utils import run_bass_kernel_spmd

    nc = _get_nc()
    return run_bass_kernel_spmd(
        nc, in_maps, core_ids=list(range(NCORES)), trace=trace, **kwargs
    )


def kernel(**inputs):
    in_maps = make_in_maps(**inputs)
    res = run(in_maps, trace=False)
    outs = [res.results[c]["out"] for c in range(NCORES)]
    return np.concatenate(outs, axis=0).reshape(B, T, D)


# revision 75
# speedup vs baseline: 1.0644x; 1.0644x over previous
"""Fused transformer block (LN1 -> 16-head causal attention -> LN2 -> FFN,
two residuals) on 8 Trainium2 NeuronCores.

Sharding strategy
-----------------
- Attention is head-parallel: core c owns heads (2c, 2c+1) and computes their
  Q^T/K^T/V^T and attention output O^T for ALL 4096 tokens (B*T flattened).
- LN1 + transpose of the normalized activations is sequence-parallel (512
  tokens per core), stitched with an AllGather of x^T (feature-major).
- An AllToAll converts the head-parallel attention output into token-parallel
  layout; residual + LN2 + the whole FFN then run sequence-parallel (512
  tokens per core) with zero further communication. Each core returns its
  512-token slice of the output; the host concatenates.

Performance notes:
- LN gains/biases are folded into the weights host-side: Wq/Wk/Wv absorb
  ln1_g (per-head bias vectors absorb ln1_b), W1 absorbs ln2_g and b1
  absorbs ln2_b@W1. LN on device is just (x - mean) * rstd.
- The gathered x^T is read back in ONE 3-D DMA per 512-token chunk; W1 is
  preloaded in 8 row-block DMAs right after the AllToAll completes; the
  output is accumulated in SBUF and written in 4 DMAs.
- Attention is software-pipelined across chunks: scores+exp of chunk c+1 are
  emitted before the PV accumulation of chunk c, so the PE always has
  runnable work while the Scalar engine drains the softmax exps. Two score
  tiles share a 2-bank PSUM tile so one Exp covers 1024 columns.
- The attention-output AllToAll travels in bf16 (halves the exchange).
"""

import sys

if "/opt/trn_rl_repo" not in sys.path:
    sys.path.insert(0, "/opt/trn_rl_repo")

import ml_dtypes
import numpy as np

import concourse.bass as bass
import concourse.mybir as mybir
import concourse.tile as tile
from concourse import bacc
from concourse.bass import ds, ts
from concourse.masks import make_identity

# ── Problem constants (hardcoded; see spec) ──────────────────────────────────
B, T, D = 2, 2048, 1024
H, HS = 16, 64
FF = 4 * D  # 4096
EPS = 1e-5
NCORES = 8
NT = B * T  # 4096 tokens
TC = NT // NCORES  # 512 tokens per core
NTT = TC // 128  # 4 token tiles per core
HPC = H // NCORES  # 2 heads per core
SCALE = 1.0 / float(np.sqrt(D))  # reference scales scores by D**-0.5

F32 = mybir.dt.float32
BF16 = mybir.dt.bfloat16
AF = mybir.ActivationFunctionType
OP = mybir.AluOpType


def build():
    nc = bacc.Bacc(num_devices=NCORES)

    emb = nc.dram_tensor("emb", [TC, D], F32, kind="ExternalInput")
    # per-core head slice of Wq/Wk/Wv (ln1_g folded in), pre-rearranged to
    # [128(d_in), 8(d_tile), 128(2*HS)]
    wq = nc.dram_tensor("wq", [128, 8, 128], BF16, kind="ExternalInput")
    wk = nc.dram_tensor("wk", [128, 8, 128], BF16, kind="ExternalInput")
    wv = nc.dram_tensor("wv", [128, 8, 128], BF16, kind="ExternalInput")
    # per-core QKV bias columns (ln1_b folded through the projections)
    bqkv = nc.dram_tensor("bqkv", [128, 3], F32, kind="ExternalInput")
    w1 = nc.dram_tensor("w1", [D, FF], BF16, kind="ExternalInput")
    w2 = nc.dram_tensor("w2", [FF, D], BF16, kind="ExternalInput")
    b1r = nc.dram_tensor("b1r", [128, FF // 128], F32, kind="ExternalInput")
    b2f = nc.dram_tensor("b2f", [D], F32, kind="ExternalInput")
    out = nc.dram_tensor("out", [TC, D], F32, kind="ExternalOutput")

    rg = [list(range(NCORES))]

    with tile.TileContext(nc) as tc:
        # Pools are allocated just-in-time and released LIFO per (space, side).
        const = tc.alloc_tile_pool(name="const", bufs=1)
        dram = tc.alloc_tile_pool(name="dram", bufs=1, space="DRAM")
        stat = tc.alloc_tile_pool(name="stat", bufs=4)
        xln = tc.alloc_tile_pool(name="xln", bufs=2)
        embs = tc.alloc_tile_pool(name="embs", bufs=2, side="right")

        identity = const.tile([128, 128], F32, name="identity")
        make_identity(nc, identity)
        identity_b = const.tile([128, 128], BF16, name="identity_b")
        nc.vector.tensor_copy(out=identity_b, in_=identity)
        eps_t = const.tile([128, 1], F32, name="eps_t")
        nc.vector.memset(eps_t, EPS)
        b1s = const.tile([128, FF // 128], F32, name="b1s")
        nc.sync.dma_start(out=b1s, in_=b1r[:, :])
        b2b = const.tile([128, D], F32, name="b2b")
        b2a = b2f[:]
        nc.sync.dma_start(
            out=b2b,
            in_=bass.AP(tensor=b2a.tensor, offset=b2a.offset, ap=[[0, 128], [1, D]]),
        )
        bqs = const.tile([128, 3], F32, name="bqs")
        nc.sync.dma_start(out=bqs, in_=bqkv[:, :])

        # DRAM bounce buffers
        cc_x_in = dram.tile([D, TC], BF16, name="cc_x_in")
        xg = dram.tile([NCORES * D, TC], BF16, name="xg", addr_space="Shared")
        cc_a_in = dram.tile([NCORES, 130, TC], BF16, name="cc_a_in")
        cc_a_out = dram.tile([NCORES, 130, TC], BF16, name="cc_a_out")

        # attention-lifetime pools (released after the AllToAll)
        qkres = tc.alloc_tile_pool(name="qkres", bufs=1)
        vsbp = tc.alloc_tile_pool(name="vsbp", bufs=1)
        otp = tc.alloc_tile_pool(name="otp", bufs=3)
        ptp = tc.alloc_tile_pool(name="ptp", bufs=6)
        attnc = tc.alloc_tile_pool(name="attnc", bufs=1)

        qT = qkres.tile([128, NT], BF16, name="qT")
        kT = qkres.tile([128, NT], BF16, name="kT")
        v_sb = vsbp.tile([128, NT // 128, HPC, 65], BF16, name="v_sb")
        ones_f = attnc.tile([128, 64], F32, name="ones_f")
        nc.vector.memset(ones_f, 1.0)

        # QKV-lifetime pools
        vtp = tc.alloc_tile_pool(name="vtp", bufs=1)
        wqkvc = tc.alloc_tile_pool(name="wqkvc", bufs=1)
        xrhs = tc.alloc_tile_pool(name="xrhs", bufs=2)
        vT = vtp.tile([128, NT], BF16, name="vT")
        wqs = wqkvc.tile([128, 8, 128], BF16, name="wqs")
        nc.sync.dma_start(out=wqs, in_=wq[:, :, :])
        wks = wqkvc.tile([128, 8, 128], BF16, name="wks")
        nc.sync.dma_start(out=wks, in_=wk[:, :, :])
        wvs = wqkvc.tile([128, 8, 128], BF16, name="wvs")
        nc.sync.dma_start(out=wvs, in_=wv[:, :, :])

        def layer_norm(src_tile, dst_tile, use_scalar=False):
            """dst = (src - mean) * rstd  (per 128-token tile, stats over D).

            LN gain/bias are folded into the downstream weights host-side.
            With use_scalar the wide normalize pass runs on the Scalar engine
            as Identity(src * rstd + (-mean * rstd)), halving the Vector load.
            """
            st = stat.tile([128, 2, 6], F32, name="st", tag="st")
            nc.vector.bn_stats(out=st[:, 0, :], in_=src_tile[:, 0:512])
            nc.vector.bn_stats(out=st[:, 1, :], in_=src_tile[:, 512:1024])
            mv = stat.tile([128, 2], F32, name="mv", tag="mv")
            nc.vector.bn_aggr(out=mv, in_=st)
            std = stat.tile([128, 1], F32, name="std", tag="std")
            nc.scalar.activation(
                out=std, in_=mv[:, 1:2], func=AF.Sqrt, bias=eps_t, scale=1.0
            )
            rstd = stat.tile([128, 1], F32, name="rstd", tag="rstd")
            nc.vector.reciprocal(out=rstd, in_=std)
            if use_scalar:
                nm = stat.tile([128, 1], F32, name="nm", tag="nm")
                nc.vector.tensor_scalar(
                    out=nm,
                    in0=mv[:, 0:1],
                    scalar1=rstd,
                    scalar2=-1.0,
                    op0=OP.mult,
                    op1=OP.mult,
                )
                nc.scalar.activation(
                    out=dst_tile, in_=src_tile, func=AF.Identity, bias=nm, scale=rstd
                )
            else:
                nc.vector.tensor_scalar(
                    out=dst_tile,
                    in0=src_tile,
                    scalar1=mv[:, 0:1],
                    scalar2=rstd,
                    op0=OP.subtract,
                    op1=OP.mult,
                )

        # ── Phase A+B: LN1 on own 512-token chunk, transpose, send to AG ──
        xtp = tc.alloc_tile_pool(name="xtp", bufs=1)
        ab_tp = tc.alloc_tile_pool(name="ab_tp", bufs=4, space="PSUM")
        xt_tiles = [xtp.tile([128, TC], BF16, name=f"xt{k}") for k in range(8)]
        xn_tiles = []
        for i in range(NTT):
            et = embs.tile([128, D], F32, name="et", tag="et", bufs=4)
            nc.sync.dma_start(out=et, in_=emb[ts(i, 128), :])
            xn = xln.tile([128, D], F32, name="xn", tag="xn", bufs=4)
            layer_norm(et, xn, use_scalar=(i % 2 == 1))
            xn_tiles.append(xn)
        # k-major transposes with a per-block DMA: each feature block of the
        # AllGather input ships as soon as it is complete, so the collective's
        # data dependency clears right after the last block instead of after
        # a bulk 8-DMA tail
        for k in range(8):
            for i in range(NTT):
                ps = ab_tp.tile([128, 128], F32, name="abtp", tag="abtp")
                nc.tensor.transpose(ps, xn_tiles[i][:, ts(k, 128)], identity)
                if k % 2 == 0:
                    nc.vector.tensor_copy(out=xt_tiles[k][:, ts(i, 128)], in_=ps)
                else:
                    nc.scalar.copy(out=xt_tiles[k][:, ts(i, 128)], in_=ps)
            nc.sync.dma_start(out=cc_x_in[ts(k, 128), :], in_=xt_tiles[k])

        # ── Phase C: AllGather x^T chunks ───────────────────────────────
        nc.gpsimd.collective_compute(
            "AllGather", OP.bypass, replica_groups=rg, ins=[cc_x_in.opt()], outs=[xg.opt()]
        )
        xtp.release()
        ab_tp.release()

        # causal masks (emitted after the collective so GpSimd reaches the
        # collective trigger as early as possible)
        masks = []
        for i in range(4):
            m = attnc.tile([128, TC], BF16, name=f"mask{i}")
            nc.gpsimd.memset(m, 1.0)
            # keep 1 where (j - i_partition - 128*i) >= 0, else 0
            nc.gpsimd.affine_select(
                out=m,
                in_=m,
                compare_op=OP.is_ge,
                fill=0.0,
                base=-128 * i,
                pattern=[[1, TC]],
                channel_multiplier=-1,
            )
            masks.append(m)

        # ── Phase D: Q^T/K^T/V^T for own 2 heads over all 4096 tokens ──
        qkv_ps = tc.alloc_tile_pool(name="qkv_ps", bufs=2, space="PSUM")
        for cb in range(NCORES):
            xr = xrhs.tile([128, 8, TC], BF16, name="xr", tag="xr")
            xga = xg[:]
            if cb == 0:
                # first chunk: per-block DMAs so the k=0 matmul starts as
                # soon as its 128 KB lands instead of after the full MB
                for k in range(8):
                    src = bass.AP(
                        tensor=xga.tensor,
                        offset=xga.offset + 128 * k * TC,
                        ap=[[TC, 128], [1, TC]],
                    )
                    nc.sync.dma_start(out=xr[:, k, :], in_=src)
            else:
                # one 3-D DMA brings in all 8 feature blocks of this chunk
                src = bass.AP(
                    tensor=xga.tensor,
                    offset=xga.offset + D * cb * TC,
                    ap=[[TC, 128], [128 * TC, 8], [1, TC]],
                )
                nc.sync.dma_start(out=xr, in_=src)
            pq = qkv_ps.tile([128, TC], F32, name="pq", tag="pq")
            pk = qkv_ps.tile([128, TC], F32, name="pk", tag="pk")
            pv = qkv_ps.tile([128, TC], F32, name="pv", tag="pv")
            for k in range(8):
                kw = dict(start=(k == 0), stop=(k == 7))
                nc.tensor.matmul(pq, lhsT=wqs[:, k, :], rhs=xr[:, k, :], **kw)
                nc.tensor.matmul(pk, lhsT=wks[:, k, :], rhs=xr[:, k, :], **kw)
                nc.tensor.matmul(pv, lhsT=wvs[:, k, :], rhs=xr[:, k, :], **kw)
            nc.vector.tensor_scalar_add(
                out=qT[:, ts(cb, TC)], in0=pq, scalar1=bqs[:, 0:1]
            )
            nc.vector.tensor_scalar_add(
                out=kT[:, ts(cb, TC)], in0=pk, scalar1=bqs[:, 1:2]
            )
            nc.scalar.activation(
                out=vT[:, ts(cb, TC)], in_=pv, func=AF.Identity,
                bias=bqs[:, 2:3], scale=1.0,
            )

        # ── Phase E: V^T -> natural V per s-tile, ones-augmented ────────
        for st_ in range(NT // 128):
            ps = qkv_ps.tile([128, 128], BF16, name="tpb", tag="tpb")
            nc.tensor.transpose(ps, vT[:, ts(st_, 128)], identity_b)
            nc.vector.tensor_copy(
                out=v_sb[:, st_, :, 0:64],
                in_=ps.rearrange("p (h e) -> p h e", h=HPC),
            )
        nc.vector.tensor_copy(
            out=v_sb[:, :, :, 64:65],
            in_=ones_f[:, 0:64].rearrange("p (a b c) -> p a b c", a=NT // 128, b=HPC),
        )
        xrhs.release()
        wqkvc.release()
        vtp.release()
        qkv_ps.release()

        # ── Phase F: causal attention, software-pipelined across chunks ──
        # Per chunk: S^T matmuls stream through PSUM banks, exp evacuates to
        # bf16 SBUF; causal masking is a binary multiply on the exp tiles.
        # The scores+exp of chunk c+1 are emitted BEFORE the PV accumulation
        # of chunk c so the PE has dependency-free work while Scalar drains.
        s_ps = tc.alloc_tile_pool(name="s_ps", bufs=3, space="PSUM")
        o_ps = tc.alloc_tile_pool(name="o_ps", bufs=1, space="PSUM")

        def emit_scores(gc):
            b = gc // 4
            lc = gc % 4
            nst = 4 * lc + 4  # s-tiles (128 wide) within this batch
            t0g = gc * TC
            pts = {}
            # two s-tiles share one 2-bank PSUM tile so a single Exp covers
            # 1024 columns (halves the Scalar per-instruction overhead)
            # Causal column-skipping: a diag-d tile's first 128*d query
            # columns never survive the mask, so every stage (scores, exp,
            # mask, PV) is restricted to the columns the next stage reads.
            for sp in range(nst // 2):
                d0 = 2 * sp - 4 * lc
                d1 = d0 + 1
                cl = 128 * d0 if d0 > 0 else 0  # pair-wide exp start column
                for h in range(HPC):
                    ps_ = s_ps.tile([128, 2, TC], F32, name="ps_", tag="ps_")
                    pt_ = ptp.tile([128, 2, TC], BF16, name="pt_", tag="pt_", bufs=30)
                    for hf in range(2):
                        stl = 2 * sp + hf
                        sg = b * 16 + stl
                        diag = stl - 4 * lc
                        c0 = 128 * diag if diag > 0 else 0
                        # heads use PE row-groups 0-63 / 64-127 -> concurrent
                        nc.tensor.matmul(
                            ps_[:, hf, ds(c0, TC - c0)],
                            lhsT=kT[ts(h, 64), ts(sg, 128)],
                            rhs=qT[ts(h, 64), ds(t0g + c0, TC - c0)],
                            start=True,
                            stop=True,
                        )
                    nc.scalar.activation(
                        out=pt_[:, :, ds(cl, TC - cl)],
                        in_=ps_[:, :, ds(cl, TC - cl)],
                        func=AF.Exp,
                        scale=SCALE,
                    )
                    for hf in range(2):
                        stl = 2 * sp + hf
                        diag = stl - 4 * lc
                        c0 = 128 * diag if diag > 0 else 0
                        if diag >= 0:
                            nc.vector.tensor_mul(
                                out=pt_[:, hf, ds(c0, TC - c0)],
                                in0=pt_[:, hf, ds(c0, TC - c0)],
                                in1=masks[diag][:, ds(c0, TC - c0)],
                            )
                        pts[(stl, h)] = (pt_, hf, c0)
            return pts

        def emit_pv(gc, pts):
            b = gc // 4
            lc = gc % 4
            nst = 4 * lc + 4
            oT = otp.tile([128, TC], BF16, name="oT", tag="oT")
            for h in range(HPC):
                po = o_ps.tile([65, TC], F32, name=f"po{h}", tag=f"po{h}", bufs=1)
                for stl in range(nst):
                    sg = b * 16 + stl
                    pt_, hf, c0 = pts[(stl, h)]
                    nc.tensor.matmul(
                        po[:, ds(c0, TC - c0)],
                        lhsT=v_sb[:, sg, h, :],
                        rhs=pt_[:, hf, ds(c0, TC - c0)],
                        start=(stl == 0),
                        stop=(stl == nst - 1),
                        skip_group_check=(c0 > 0),
                    )
                nc.vector.tensor_copy(out=oT[ts(h, 64), :], in_=po[0:64, :])
                oTd = otp.tile([1, TC], BF16, name="oTd", tag="oTd")
                nc.vector.tensor_copy(out=oTd, in_=po[64:65, :])
                nc.sync.dma_start(out=cc_a_in[gc, 128 + h : 129 + h, :], in_=oTd)
            nc.sync.dma_start(out=cc_a_in[gc, 0:128, :], in_=oT)

        # residual emb tiles for phase H: loaded during attention so the
        # DMAs don't contend with the AllToAll
        et_tiles = []
        for i in range(NTT):
            et = embs.tile([128, D], F32, name="et", tag=f"et{i}", bufs=1)
            nc.sync.dma_start(out=et, in_=emb[ts(i, 128), :])
            et_tiles.append(et)

        prev = None
        for gc in range(NCORES):
            pts = emit_scores(gc)
            if prev is not None:
                emit_pv(prev[0], prev[1])
            prev = (gc, pts)
        emit_pv(prev[0], prev[1])

        # ── Phase G: AllToAll -> unnormalized attn^T + denoms, own tokens ──
        nc.gpsimd.collective_compute(
            "AllToAll", OP.bypass, replica_groups=rg, ins=[cc_a_in.opt()], outs=[cc_a_out.opt()]
        )
        o_ps.release()
        s_ps.release()
        attnc.release()
        ptp.release()
        otp.release()
        vsbp.release()
        qkres.release()

        # ── Phase H: normalize + attn residual + LN2, y -> y^T ──────────
        asbp = tc.alloc_tile_pool(name="asbp", bufs=4, side="right")
        h_tp = tc.alloc_tile_pool(name="h_tp", bufs=5, space="PSUM")
        x2p = tc.alloc_tile_pool(name="x2p", bufs=1)
        ytp = tc.alloc_tile_pool(name="ytp", bufs=1)
        w1p = tc.alloc_tile_pool(name="w1p", bufs=1)
        x2_tiles = [x2p.tile([128, D], F32, name=f"x2_{i}") for i in range(NTT)]
        yt_tiles = [ytp.tile([128, TC], BF16, name=f"yt{k}") for k in range(8)]
        w1sb = [w1p.tile([128, FF], BF16, name=f"w1sb{k}") for k in range(8)]
        asb_tiles = []
        for c in range(NCORES):
            asb = asbp.tile([128, TC], BF16, name="asb", tag="asb", bufs=8)
            nc.sync.dma_start(out=asb, in_=cc_a_out[c, 0:128, :])
            dnm = asbp.tile([2, TC], BF16, name="dnm", tag="dnm", bufs=8)
            nc.sync.dma_start(out=dnm, in_=cc_a_out[c, 128:130, :])
            asb_tiles.append((asb, dnm))
            if c == 0:
                # W1 preload: issued on Sync right after the first post-A2A
                # load so the 8 MB of reads don't contend with the collective
                for k in range(8):
                    nc.sync.dma_start(out=w1sb[k], in_=w1[ts(k, 128), :])
        # i-major: each x2 tile completes as early as possible so its LN2
        # (emitted right after) overlaps the remaining residual work
        yn_tiles = []
        for i in range(NTT):
            for c in range(NCORES):
                asb, dnm = asb_tiles[c]
                pn = h_tp.tile([128, 128], BF16, name="htp", tag="htp")
                nc.tensor.transpose(pn, asb[:, ts(i, 128)], identity_b)
                pd = h_tp.tile([128, 2], BF16, name="hpd", tag="htp")
                nc.tensor.transpose(pd, dnm[:, ts(i, 128)], identity_b[0:2, 0:2])
                rcp = asbp.tile([128, 2], F32, name="rcp", tag="rcp")
                nc.vector.reciprocal(out=rcp, in_=pd)
                for h in range(HPC):
                    nc.vector.scalar_tensor_tensor(
                        out=x2_tiles[i][:, ds(128 * c + 64 * h, 64)],
                        in0=pn[:, ts(h, 64)],
                        scalar=rcp[:, h : h + 1],
                        in1=et_tiles[i][:, ds(128 * c + 64 * h, 64)],
                        op0=OP.mult,
                        op1=OP.add,
                    )
            yn = xln.tile([128, D], F32, name="yn", tag="yn", bufs=4)
            layer_norm(x2_tiles[i], yn, use_scalar=(i % 2 == 1))
            yn_tiles.append(yn)
        # k-major transposes: yt[0] (which gates the FFN's first matmul)
        # completes first instead of last
        for k in range(8):
            for i in range(NTT):
                ps = h_tp.tile([128, 128], F32, name="htp2", tag="htp2", bufs=3)
                nc.tensor.transpose(ps, yn_tiles[i][:, ts(k, 128)], identity)
                if k % 2 == 0:
                    nc.vector.tensor_copy(out=yt_tiles[k][:, ts(i, 128)], in_=ps)
                else:
                    nc.scalar.copy(out=yt_tiles[k][:, ts(i, 128)], in_=ps)
        asbp.release()
        embs.release()
        h_tp.release()

        # ── Phase J: FFN up-projection, h^T = relu(W1^T y^T + b1) ───────
        htp = tc.alloc_tile_pool(name="htp", bufs=1)
        w2sp = tc.alloc_tile_pool(name="w2sp", bufs=6)
        outsp = tc.alloc_tile_pool(name="outs", bufs=1)
        h_ps = tc.alloc_tile_pool(name="h_ps", bufs=4, space="PSUM")
        ht_tiles = [htp.tile([128, TC], BF16, name=f"ht{j}") for j in range(FF // 128)]
        out_sb = [outsp.tile([128, D], F32, name=f"osb{i}") for i in range(NTT)]
        # fold the down-projection bias into the residual while Vector is idle
        # (LN2 has already consumed x2, so this is safe)
        for i in range(NTT):
            nc.vector.tensor_add(out=x2_tiles[i], in0=x2_tiles[i], in1=b2b)
        for jg in range(16):
            phs = [h_ps.tile([128, TC], F32, name=f"ph{jj}", tag="ph") for jj in range(2)]
            for k in range(8):
                for jj in range(2):
                    nc.tensor.matmul(
                        phs[jj],
                        lhsT=w1sb[k][:, ds(256 * jg + 128 * jj, 128)],
                        rhs=yt_tiles[k],
                        start=(k == 0),
                        stop=(k == 7),
                    )
            for jj in range(2):
                jt = 2 * jg + jj
                nc.scalar.activation(
                    out=ht_tiles[jt],
                    in_=phs[jj],
                    func=AF.Relu,
                    bias=b1s[:, jt : jt + 1],
                    scale=1.0,
                )
        h_ps.release()

        # ── Phase K: FFN down-projection, natural [token, D] accumulation ──
        # lhsT is an h^T chunk reused for both 512-wide halves of W2's rows;
        # each token tile owns a 2-bank PSUM accumulator, so the output needs
        # no final transposes — just one residual add per tile.
        f_ps = tc.alloc_tile_pool(name="f_ps", bufs=4, space="PSUM")
        pfs = [f_ps.tile([128, D], F32, name=f"pf{i}", tag="pf") for i in range(NTT)]
        for jt in range(FF // 128):
            w2t = w2sp.tile([128, D], BF16, name="w2t", tag="w2t")
            nc.sync.dma_start(out=w2t, in_=w2[ts(jt, 128), :])
            for i in range(NTT):
                for dh in range(2):
                    nc.tensor.matmul(
                        pfs[i][:, ts(dh, 512)],
                        lhsT=ht_tiles[jt][:, ts(i, 128)],
                        rhs=w2t[:, ts(dh, 512)],
                        start=(jt == 0),
                        stop=(jt == FF // 128 - 1),
                    )
        for i in range(NTT):
            # half-width adds + writes so the output DMA starts draining
            # while the second half is still being summed
            for dh in range(2):
                nc.vector.tensor_add(
                    out=out_sb[i][:, ts(dh, 512)],
                    in0=pfs[i][:, ts(dh, 512)],
                    in1=x2_tiles[i][:, ts(dh, 512)],
                )
                nc.sync.dma_start(
                    out=out[ts(i, 128), ds(512 * dh, 512)],
                    in_=out_sb[i][:, ts(dh, 512)],
                )

        f_ps.release()
        outsp.release()
        w2sp.release()
        htp.release()
        w1p.release()
        ytp.release()
        x2p.release()
        xln.release()
        stat.release()
        dram.release()
        const.release()
    nc.finalize()
    return nc


_NC = None


def _get_nc():
    global _NC
    if _NC is None:
        _NC = build()
    return _NC


def make_in_maps(embds, Wq, Wk, Wv, ln1_g, ln1_b, ln2_g, ln2_b, W1, b1, W2, b2):
    embds = np.ascontiguousarray(np.asarray(embds, dtype=np.float32)).reshape(NT, D)
    Wq = np.asarray(Wq, dtype=np.float32)
    Wk = np.asarray(Wk, dtype=np.float32)
    Wv = np.asarray(Wv, dtype=np.float32)
    W1 = np.ascontiguousarray(np.asarray(W1, dtype=np.float32))
    W2 = np.ascontiguousarray(np.asarray(W2, dtype=np.float32))
    b1 = np.asarray(b1, dtype=np.float32)
    b2 = np.asarray(b2, dtype=np.float32)
    g1 = np.asarray(ln1_g, dtype=np.float32)
    bb1 = np.asarray(ln1_b, dtype=np.float32)
    g2 = np.asarray(ln2_g, dtype=np.float32)
    bb2 = np.asarray(ln2_b, dtype=np.float32)

    # Fold LN1 gain/bias into the QKV projections:
    #   q = (xn*g1 + b1) @ Wq = xn @ (g1[:,None]*Wq) + b1@Wq
    Wqf = Wq * g1[None, :, None]
    Wkf = Wk * g1[None, :, None]
    Wvf = Wv * g1[None, :, None]
    bq = np.einsum("d,hde->he", bb1, Wq)  # [H, HS]
    bk = np.einsum("d,hde->he", bb1, Wk)
    bv = np.einsum("d,hde->he", bb1, Wv)

    # Fold LN2 gain/bias into the FFN up-projection:
    #   h_pre = (yn*g2 + b2ln) @ W1 + b1 = yn @ (g2[:,None]*W1) + (b2ln@W1 + b1)
    W1f = (W1 * g2[:, None]).astype(ml_dtypes.bfloat16)
    b1f = b1 + bb2 @ W1
    W2b = W2.astype(ml_dtypes.bfloat16)
    b1r = np.ascontiguousarray(b1f.reshape(FF // 128, 128).T.astype(np.float32))

    def _w_slice(W, c):
        # heads (2c, 2c+1): [2, D, HS] -> [D, 2*HS] -> [128, 8, 128]
        s = W[2 * c : 2 * c + 2].transpose(1, 0, 2).reshape(D, 2 * HS)
        r = np.ascontiguousarray(s.reshape(8, 128, 2 * HS).transpose(1, 0, 2))
        return r.astype(ml_dtypes.bfloat16)

    in_maps = []
    for c in range(NCORES):
        bqkv = np.stack(
            [
                np.concatenate([bq[2 * c], bq[2 * c + 1]]),
                np.concatenate([bk[2 * c], bk[2 * c + 1]]),
                np.concatenate([bv[2 * c], bv[2 * c + 1]]),
            ],
            axis=1,
        ).astype(np.float32)  # [128, 3]
        in_maps.append(
            {
                "emb": np.ascontiguousarray(embds[c * TC : (c + 1) * TC]),
                "wq": _w_slice(Wqf, c),
                "wk": _w_slice(Wkf, c),
                "wv": _w_slice(Wvf, c),
                "bqkv": np.ascontiguousarray(bqkv),
                "w1": W1f,
                "w2": W2b,
                "b1r": b1r,
                "b2f": np.ascontiguousarray(b2),
            }
        )
    return in_maps


def run(in_maps, trace=False, **kwargs):
    from concourse.bass_utils import run_bass_kernel_spmd

    nc = _get_nc()
    return run_bass_kernel_spmd(
        nc, in_maps, core_ids=list(range(NCORES)), trace=trace, **kwargs
    )


def kernel(**inputs):
    in_maps = make_in_maps(**inputs)
    res = run(in_maps, trace=False)
    outs = [res.results[c]["out"] for c in range(NCORES)]
    return np.concatenate(outs, axis=0).reshape(B, T, D)
